# revision 29
# baseline (speedup 1.0000x reference)
"""Trainium2 Bass kernel for nn_EnhancedBioKANModel (dense_transformer).

Model (B=4096, IN=3072, D=2048, C=1000, 3 blocks), with the key
mathematical simplifications:

1. The internal sequence length is 1, so attention's softmax over a single
   key is identically 1.0 and the whole score/top-k/conv machinery cancels:
       attn(h) = (h @ Wv^T + bv) @ Wo^T + bo
2. That attention is then a purely linear map, so the residual attention
   block folds into ONE matrix host-side:
       h <- h + attn(h) = (I + Wo Wv) h + (Wo bv + bo) = Wff h + bf
3. The MLP branch is decoupled from the attention output by pre-multiplying
   host-side:  W1A = W1c @ Wff, so  s = W1c h_nxt == W1A h + const.
   Per layer the device then runs   W1A -> W2 -> Wff   and the whole
   LayerNorm scale chain + residual applies overlap with Wff's PE work
   (no PE idle at layer boundaries).

Strategy: pure data-parallel over batch (512 rows/core on 8 cores),
feature-major activation layout [D partitions, 512 batch free],
host-pre-transposed fp16 weights (halves HBM traffic vs fp32 — the fp32
version is DMA-bound at ~290 GB/s/core), fp16 activations (matmul is
1 col/cycle for fp16 and fp32r alike; PSUM accumulates fp32), LayerNorm
means folded into host-centered W1/W2, variance via ScalarE-square + PE
ones-reduction, per-batch 1/std broadcast via K=1 matmul, PE warm-up
burst at start so the HAM clock gate is at 8/8 when the real GEMM
stream begins.
"""
from contextlib import ExitStack

import numpy as np

import concourse.bacc as bacc
import concourse.mybir as mybir
import concourse.tile as tile
from concourse.bass_utils import run_bass_kernel_spmd

F32 = mybir.dt.float32
F16 = mybir.dt.float16
AF = mybir.ActivationFunctionType
ALU = mybir.AluOpType

NCORES = 8
B, IN, D, DD, C = 4096, 3072, 2048, 4096, 1000
CP = 1024          # padded num_classes
BC = B // NCORES   # 512 batch per core
NB = 3
EPS = 1e-5
MG = 2             # m-chunks per psum group (2 -> zero group-boundary stall)
KB = 8             # k-chunks per weight DMA (512 KiB transfers, 4 KiB lines)
NWARM = 120        # PE warm-up matmuls: cover the DMA-ring ramp (~8-18us)
SQR = 8            # sq chunks pre-reduced on DVE per variance matmul

# consts packing (columns of [128, NCOL] fp32), per layer:
#   bf(16) b1c(32) g1(32) beta1(32) b2c(16) g2(16) beta2(16)
_LAYER_COLS = 160
_NCOL = 16 + NB * _LAYER_COLS + CP // 128

_cached = None
last_results = None


def _build(fast=True):
    nc = bacc.Bacc(trn_type="TRN2")

    def wparam(name, K, M):
        # swizzled: [M/256 groups, K/(128*KB) blocks, 128 partitions, KB*256]
        return nc.declare_dram_parameter(
            name, [M // (MG * 128), K // (128 * KB), 128, KB * MG * 128],
            F16, isOutput=False)

    xt = nc.declare_dram_parameter("xt", [IN, BC], F16, isOutput=False)
    wit = wparam("wit", IN, D)
    w1t = [wparam(f"w1t{l}", D, DD) for l in range(NB)]
    w2t = [wparam(f"w2t{l}", DD, D) for l in range(NB)]
    wfft = [wparam(f"wfft{l}", D, D) for l in range(NB)]
    woutt = wparam("woutt", D, CP)
    consts = nc.declare_dram_parameter("consts", [128, _NCOL], F32, isOutput=False)
    out_t = nc.declare_dram_parameter("outT", [CP, BC], F16, isOutput=True)

    with tile.TileContext(nc) as tc, ExitStack() as ctx:
        wpool = ctx.enter_context(tc.tile_pool(name="w", bufs=4))
        xpool = ctx.enter_context(tc.tile_pool(name="x", bufs=1))
        spool = ctx.enter_context(tc.tile_pool(name="s", bufs=1))
        hapool = ctx.enter_context(tc.tile_pool(name="ha", bufs=1))
        hbpool = ctx.enter_context(tc.tile_pool(name="hb", bufs=1))
        hnpool = ctx.enter_context(tc.tile_pool(name="hn", bufs=1))
        upool = ctx.enter_context(tc.tile_pool(name="u", bufs=1))
        tpool = ctx.enter_context(tc.tile_pool(name="t", bufs=4))
        sqpool = ctx.enter_context(tc.tile_pool(name="sq", bufs=3))
        opool = ctx.enter_context(tc.tile_pool(name="o", bufs=2))
        statpool = ctx.enter_context(tc.tile_pool(name="stat", bufs=1))
        singles = ctx.enter_context(tc.tile_pool(name="singles", bufs=1))
        psum = ctx.enter_context(tc.tile_pool(name="psum", bufs=6, space="PSUM"))
        pstat = ctx.enter_context(tc.tile_pool(name="pstat", bufs=1, space="PSUM"))
        pbc = ctx.enter_context(tc.tile_pool(name="pbc", bufs=1, space="PSUM"))

        # ---- optional PE warm-up (NWARM matmuls on a memset tile; with the
        # x/weight DMAs on parallel HWDGE rings the real Wi stream starts
        # early enough to warm the HAM clock gate itself).
        if NWARM:
            wu_w = singles.tile([128, 128], F16)
            nc.vector.memset(wu_w, 0.5)
            # ping-pong two PSUM banks so the warm-up matmuls run
            # back-to-back (~100% PE duty — keeps the HAM gate warm)
            wu_a = psum.tile([128, 128], F32, name="wua", tag="ps")
            wu_b = pbc.tile([128, 128], F32, name="wub", tag="bc")
            for i in range(NWARM):
                t = wu_a if i % 2 == 0 else wu_b
                nc.tensor.matmul(t[:, :], lhsT=wu_w[:, :], rhs=wu_w[:, :],
                                 start=True, stop=True)

        # ---- constants ----
        cst = singles.tile([128, _NCOL], F32)
        nc.gpsimd.dma_start(out=cst, in_=consts[:, :])

        ones_f = singles.tile([128, 1], F32)
        nc.vector.memset(ones_f, 1.0)
        ones_col = singles.tile([128, 1], F16)
        nc.vector.tensor_copy(ones_col[:, :], ones_f[:, :])
        ones_row_f = singles.tile([1, 128], F32)
        nc.vector.memset(ones_row_f, 1.0)
        ones_row = singles.tile([1, 128], F16)
        nc.vector.tensor_copy(ones_row[:, :], ones_row_f[:, :])
        eps_sb = singles.tile([1, 1], F32)
        nc.vector.memset(eps_sb, EPS)
        eps2_sb = singles.tile([1, 1], F32)
        nc.vector.memset(eps2_sb, EPS * EPS)

        col = [0]

        def take_cols(n):
            c0 = col[0]
            col[0] += n
            return cst[:, c0:c0 + n]

        bi_v = take_cols(D // 128)
        layer_cols = []
        for l in range(NB):
            layer_cols.append(dict(
                bf=take_cols(D // 128),
                b1c=take_cols(DD // 128), g1=take_cols(DD // 128),
                beta1=take_cols(DD // 128), b2c=take_cols(D // 128),
                g2=take_cols(D // 128), beta2=take_cols(D // 128)))
        bout_v = take_cols(CP // 128)

        # fp16 copies of the per-layer apply constants (keeps the DVE
        # applies in all-16-bit mode)
        csth = singles.tile([128, NB * 2 * (D // 128)], F16)
        for l in range(NB):
            o = l * 2 * (D // 128)
            nc.vector.tensor_copy(csth[:, o:o + D // 128], layer_cols[l]["g2"])
            nc.vector.tensor_copy(csth[:, o + D // 128:o + 2 * (D // 128)],
                                  layer_cols[l]["beta2"])

        def g2h(l):
            return csth[:, l * 2 * (D // 128):l * 2 * (D // 128) + D // 128]

        def beta2h(l):
            o = l * 2 * (D // 128) + D // 128
            return csth[:, o:o + D // 128]

        # Deferred variance-reduction matmuls: the PE executes its queue in
        # order, so a stat matmul emitted right at its group's end stalls the
        # PE ~0.5us on the ScalarE Square. Instead the Square is emitted at
        # evac time and the ones-matmul is queued here, flushed a couple of
        # k-chunks into the NEXT psum group (by which time sq is ready).
        pending_stats = []

        def flush_stats():
            while pending_stats:
                ps_var, sq, m, mc = pending_stats.pop(0)
                nc.tensor.matmul(ps_var[:, :], lhsT=ones_col[:, :],
                                 rhs=sq[:, :], start=(m == 0),
                                 stop=(m == mc - 1))

        # ---- generic GEMM driver (swizzled fp16 weights, 512 KiB DMAs) ----
        def gemm(wt_dram, k_chunks, m_chunks, rhs_fn, evac_fn, label,
                 first=False, mid=None):
            """psum[m] = sum_k WT[k,m].T @ rhs(k); evac_fn(m, psum).
            mid() is emitted into the PE stream after group 1's k-loop."""
            n_groups = m_chunks // MG
            nkb = k_chunks // KB
            for mg in range(n_groups):
                mlo = mg * MG
                pss = [psum.tile([128, BC], F32, name=f"ps_{label}_{mlo + i}",
                                 tag="ps") for i in range(MG)]
                for kbi in range(nkb):
                    w_sb = wpool.tile([128, KB, MG * 128], F16,
                                      name=f"w_{label}_{mg}_{kbi}", tag="w")
                    # During the Wi phase the DMA subsystem is still ramping;
                    # split its blocks across the HWDGE (SP) and SWDGE
                    # (gpsimd) descriptor paths so both deliver in parallel.
                    weng = nc.gpsimd if (first and mg % 2 == 1) else nc.sync
                    weng.dma_start(out=w_sb, in_=wt_dram[mg, kbi])
                    for kk in range(KB):
                        k = kbi * KB + kk
                        rhs = rhs_fn(k)
                        for i in range(MG):
                            nc.tensor.matmul(
                                pss[i][:, :],
                                lhsT=w_sb[:, kk, i * 128:(i + 1) * 128],
                                rhs=rhs, start=(k == 0),
                                stop=(k == k_chunks - 1))
                        if kbi == 0 and kk == 5:
                            flush_stats()
                        if mid is not None and mg == 1 and kbi == nkb - 1 \
                                and kk == KB - 1:
                            mid()
                for i in range(MG):
                    evac_fn(mlo + i, pss[i])

        # ---- LN helpers ----
        sq_quad = []

        def sq_reduce(ps, bias_col, m, m_chunks, ps_var, label):
            """ps_var[1,BC] += colsum of (psum + bias)^2. Square on ScalarE;
            SQR chunks are tree-summed on DVE so only m_chunks/SQR
            ones-matmuls hit the PE (each costs ~0.4us of PE time);
            those matmuls are deferred into a later psum group's k-loop."""
            sq = sqpool.tile([128, BC], F16, name=f"sq_{label}_{m}", tag="sq",
                             bufs=SQR + 4)
            nc.scalar.activation(out=sq[:, :], in_=ps[:, :], func=AF.Square,
                                 bias=bias_col, scale=1.0)
            sq_quad.append(sq)
            if len(sq_quad) == SQR:
                level = list(sq_quad)
                sq_quad.clear()
                gen = 0
                while len(level) > 1:
                    nxt = []
                    for j in range(0, len(level), 2):
                        t = sqpool.tile([128, BC], F16,
                                        name=f"sqr_{label}_{m}_{gen}_{j}",
                                        tag=f"sqr{gen}_{j}", bufs=2)
                        nc.vector.scalar_tensor_tensor(
                            out=t[:, :], in0=level[j][:, :], scalar=0.0,
                            in1=level[j + 1][:, :], op0=ALU.add, op1=ALU.add)
                        nxt.append(t)
                    level = nxt
                    gen += 1
                q = m // SQR
                pending_stats.append((ps_var, level[0], q, m_chunks // SQR))

        def rsqrt_bcast(v_ap, label):
            """[1,BC] f32 v -> broadcast [128,BC] fp16 of 1/sqrt(v) in SBUF."""
            inv = statpool.tile([1, BC], F32, name=f"inv_{label}", tag="inv")
            try:
                nc.vector.reciprocal_approx_fast(out=inv[:, :], in_=v_ap)
            except AttributeError:
                with nc.allow_low_precision(reason="LN scale"):
                    nc.vector.reciprocal(out=inv[:, :], in_=v_ap)
            invh = statpool.tile([1, BC], F16, name=f"invh_{label}", tag="invh")
            nc.vector.tensor_copy(invh[:, :], inv[:, :])
            ibc = pbc.tile([128, BC], F32, name=f"ibc_{label}", tag="bc")
            nc.tensor.matmul(ibc[:, :], lhsT=ones_row[:, :], rhs=invh[:, :],
                             start=True, stop=True)
            ibc_sb = statpool.tile([128, BC], F16, name=f"ibcsb_{label}",
                                   tag="ibcsb", bufs=2)
            nc.scalar.activation(out=ibc_sb[:, :], in_=ibc[:, :],
                                 func=AF.Identity, bias=0.0, scale=1.0)
            return ibc_sb

        # ---- phase 1: h = relu(Wi @ x + bi) ----
        x_sb = [xpool.tile([128, BC], F16, name=f"x{k}", tag="xs", bufs=IN // 128)
                for k in range(IN // 128)]
        x_loaded = [False] * (IN // 128)

        def x_chunk(k):
            if not x_loaded[k]:
                # ACT-ring HWDGE: keeps the x chunks off the SP ring that
                # streams the (much larger) weight blocks
                nc.scalar.dma_start(out=x_sb[k], in_=xt[k * 128:(k + 1) * 128, :])
                x_loaded[k] = True
            return x_sb[k][:, :]

        h_a = [hapool.tile([128, BC], F16, name=f"h_a{m}", tag="ha", bufs=16)
               for m in range(D // 128)]
        h_b = [hbpool.tile([128, BC], F16, name=f"h_b{m}", tag="hb", bufs=16)
               for m in range(D // 128)]
        hn = [hnpool.tile([128, BC], F16, name=f"hn{m}", tag="hn", bufs=16)
              for m in range(D // 128)]
        s_sb = [spool.tile([128, BC], F16, name=f"s{m}", tag="s", bufs=DD // 128)
                for m in range(DD // 128)]
        u_sb = [upool.tile([128, BC], F16, name=f"u{m}", tag="um", bufs=16)
                for m in range(D // 128)]

        def evac_h0(m, ps):
            nc.scalar.activation(out=h_a[m][:, :], in_=ps[:, :], func=AF.Relu,
                                 bias=bi_v[:, m:m + 1], scale=1.0)

        gemm(wit, IN // 128, D // 128, x_chunk, evac_h0, "wi", first=True)

        # ---- phase 2: layers ----
        h_cur = h_a
        h_new = h_b
        for l in range(NB):
            lc = layer_cols[l]

            # (a) s-branch first: shat = W1A h + b1eff  (W1A = W1c @ Wff);
            #     variance stats on the fly; fast path stores relu(shat)
            #     (LN1 1/std deferred through W2 into c12)
            ps_var1 = pstat.tile([1, BC], F32, name=f"pv1_{l}", tag="pv")

            def evac_s(m, ps, lc=lc, ps_var1=ps_var1, l=l):
                sq_reduce(ps, lc["b1c"][:, m:m + 1], m, DD // 128, ps_var1,
                          f"l1_{l}")
                nc.scalar.activation(out=s_sb[m][:, :], in_=ps[:, :],
                                     func=AF.Relu if fast else AF.Identity,
                                     bias=lc["b1c"][:, m:m + 1], scale=1.0)

            gemm(w1t[l], D // 128, DD // 128,
                 lambda k, h=h_cur: h[k][:, :], evac_s, f"w1{l}")

            if fast:
                # E = eps * std1^2 = eps*(pv1/DD + eps)
                e_t = statpool.tile([1, BC], F32, name=f"e_{l}", tag="e_t")
                nc.scalar.activation(out=e_t[:, :], in_=ps_var1[:, :],
                                     func=AF.Identity, bias=eps2_sb[:, :],
                                     scale=EPS / DD)
            else:
                # explicit LN1: s = relu(shat * g1 * inv1 + beta1).
                # pv1's final stat matmuls must precede the bcast matmul in
                # the in-order PE queue.
                flush_stats()
                v1 = statpool.tile([1, BC], F32, name=f"v1_{l}", tag="e_t")
                nc.scalar.activation(out=v1[:, :], in_=ps_var1[:, :],
                                     func=AF.Sqrt, bias=eps_sb[:, :],
                                     scale=1.0 / DD)
                ibc1_sb = rsqrt_bcast(v1[:, :], f"l1_{l}")
                for m in range(DD // 128):
                    t1 = tpool.tile([128, BC], F16, name=f"t1_{l}_{m}", tag="ta")
                    nc.vector.scalar_tensor_tensor(
                        out=t1[:, :], in0=s_sb[m][:, :],
                        scalar=lc["g1"][:, m:m + 1], in1=ibc1_sb[:, :],
                        op0=ALU.mult, op1=ALU.mult)
                    nc.scalar.activation(out=s_sb[m][:, :], in_=t1[:, :],
                                         func=AF.Relu,
                                         bias=lc["beta1"][:, m:m + 1],
                                         scale=1.0)

            # (b) u = W2c s + b2c (pre-centered, |g1| folded in); var stats
            ps_var2 = pstat.tile([1, BC], F32, name=f"pv2_{l}", tag="pv")

            def evac_u(m, ps, lc=lc, ps_var2=ps_var2, l=l):
                sq_reduce(ps, lc["b2c"][:, m:m + 1], m, D // 128, ps_var2,
                          f"l2_{l}")
                nc.scalar.activation(out=u_sb[m][:, :], in_=ps[:, :],
                                     func=AF.Identity,
                                     bias=lc["b2c"][:, m:m + 1], scale=1.0)

            gemm(w2t[l], DD // 128, D // 128,
                 lambda k, s=s_sb: s[k][:, :], evac_u, f"w2{l}")

            # (c) LN2 scale: c12 = 1/sqrt(pv2/D + E)  (fast: E = eps*std1^2,
            #     recovering the deferred LN1 scale; slow: E = eps).
            #     Emitted via the Wff gemm's `mid` hook so the bcast matmul
            #     enters the in-order PE queue after Wff group 1 — by then
            #     pv2 is complete and the DVE/ScalarE chain has run, so
            #     nothing stalls and the applies overlap the Wff GEMM.
            ibc2_holder = []

            def mid_c12(l=l, ps_var2=ps_var2):
                if fast:
                    v2 = statpool.tile([1, BC], F32, name=f"v2_{l}", tag="v2")
                    nc.vector.scalar_tensor_tensor(
                        out=v2[:, :], in0=ps_var2[:, :], scalar=1.0 / D,
                        in1=e_t[:, :], op0=ALU.mult, op1=ALU.add)
                    sr = statpool.tile([1, BC], F32, name=f"sr_{l}", tag="std")
                    nc.scalar.activation(out=sr[:, :], in_=v2[:, :],
                                         func=AF.Sqrt, bias=0.0, scale=1.0)
                else:
                    sr = statpool.tile([1, BC], F32, name=f"sr_{l}", tag="std")
                    nc.scalar.activation(out=sr[:, :], in_=ps_var2[:, :],
                                         func=AF.Sqrt, bias=eps_sb[:, :],
                                         scale=1.0 / D)
                ibc2_holder.append(rsqrt_bcast(sr[:, :], f"l2_{l}"))

            # (d) attention branch: hn = Wff h + bf  (identity folded in)
            def evac_att(m, ps, lc=lc):
                nc.scalar.activation(out=hn[m][:, :], in_=ps[:, :],
                                     func=AF.Identity,
                                     bias=lc["bf"][:, m:m + 1], scale=1.0)

            gemm(wfft[l], D // 128, D // 128,
                 lambda k, h=h_cur: h[k][:, :], evac_att, f"wff{l}",
                 mid=mid_c12)
            ibc2_sb = ibc2_holder[0]

            # (e) residual: h_new = hn + (u*g2)*c12 + beta2  (all-fp16 DVE,
            #     overlapped with the Wff GEMM / next-layer W1A)
            for m in range(D // 128):
                t = tpool.tile([128, BC], F16, name=f"t_{l}_{m}", tag="ta")
                nc.vector.scalar_tensor_tensor(
                    out=t[:, :], in0=u_sb[m][:, :],
                    scalar=g2h(l)[:, m:m + 1], in1=ibc2_sb[:, :],
                    op0=ALU.mult, op1=ALU.mult)
                nc.vector.scalar_tensor_tensor(
                    out=h_new[m][:, :], in0=t[:, :],
                    scalar=beta2h(l)[:, m:m + 1], in1=hn[m][:, :],
                    op0=ALU.add, op1=ALU.add)

            h_cur, h_new = h_new, h_cur

        # ---- phase 3: outT = Wout h + bout ----
        def evac_out(m, ps):
            o_sb = opool.tile([128, BC], F16, name=f"o{m}", tag="o")
            nc.scalar.activation(out=o_sb[:, :], in_=ps[:, :], func=AF.Identity,
                                 bias=bout_v[:, m:m + 1], scale=1.0)
            nc.sync.dma_start(out=out_t[m * 128:(m + 1) * 128, :], in_=o_sb[:, :])

        gemm(woutt, D // 128, CP // 128,
             lambda k, h=h_cur: h[k][:, :], evac_out, "wout")

    nc.compile()
    return nc


def _vec_cols(v):
    v = np.ascontiguousarray(v, dtype=np.float32)
    return v.reshape(-1, 128).T  # [128, L/128]


def _swizzle_w(wt):
    """[K, M] f16 -> [M/256, K/(128*KB), 128, KB*256] so one DMA moves a
    contiguous 512 KiB block with 4 KiB per-partition lines."""
    K, M = wt.shape
    MW = MG * 128
    a = wt.reshape(K // (128 * KB), KB, 128, M // MW, MW)
    return np.ascontiguousarray(a.transpose(3, 0, 2, 1, 4).reshape(
        M // MW, K // (128 * KB), 128, KB * MW))


def _prep(x, Wi, bi, Wv, bv, Wo, bo, W1, b1, ln1_g, ln1_b,
          W2, b2, ln2_g, ln2_b, Wout, bout, fast=True):
    f = np.float64
    xt_all = np.ascontiguousarray(np.asarray(x, np.float16).T)   # [IN, B]
    wit = _swizzle_w(np.asarray(Wi, np.float32).T.astype(np.float16))

    shared = {"wit": wit, "consts": None}
    consts_cols = [_vec_cols(np.asarray(bi, np.float32))]
    eye = np.eye(D, dtype=f)
    for l in range(NB):
        Wvl = np.asarray(Wv[l], f)
        Wol = np.asarray(Wo[l], f)
        bvl = np.asarray(bv[l], f)
        bol = np.asarray(bo[l], f)
        Wff = eye + Wol @ Wvl
        bf = Wol @ bvl + bol
        W1l = np.asarray(W1[l], f)
        W2l = np.asarray(W2[l], f)
        W1c = W1l - W1l.mean(axis=0, keepdims=True)
        W2c = W2l - W2l.mean(axis=0, keepdims=True)
        b1l = np.asarray(b1[l], f)
        b2l = np.asarray(b2[l], f)
        b1cl = b1l - b1l.mean()
        if fast:
            # fold sign(g1) into W1 rows (so relu-at-evac is valid) and
            # |g1| into W2 columns; requires ln1_b == 0.
            g1l = np.asarray(ln1_g[l], f)
            sgn = np.where(g1l < 0, -1.0, 1.0)
            W1c = W1c * sgn[:, None]
            b1cl = b1cl * sgn
            W2c = W2c * np.abs(g1l)[None, :]
        # decouple the s-branch from the attention output: s = W1c h_nxt
        # == (W1c Wff) h + (W1c bf + b1c)
        W1A = W1c @ Wff
        b1eff = W1c @ bf + b1cl
        shared[f"wfft{l}"] = _swizzle_w(
            np.ascontiguousarray(Wff.T).astype(np.float16))
        shared[f"w1t{l}"] = _swizzle_w(
            np.ascontiguousarray(W1A.T).astype(np.float16))
        shared[f"w2t{l}"] = _swizzle_w(
            np.ascontiguousarray(W2c.T).astype(np.float16))
        consts_cols += [
            _vec_cols(bf),
            _vec_cols(b1eff), _vec_cols(np.asarray(ln1_g[l], np.float32)),
            _vec_cols(np.asarray(ln1_b[l], np.float32)),
            _vec_cols(b2l - b2l.mean()),
            _vec_cols(np.asarray(ln2_g[l], np.float32)),
            _vec_cols(np.asarray(ln2_b[l], np.float32))]
    wout_pad = np.zeros((CP, D), np.float32)
    wout_pad[:C] = np.asarray(Wout, np.float32)
    bout_pad = np.zeros((CP,), np.float32)
    bout_pad[:C] = np.asarray(bout, np.float32)
    shared["woutt"] = _swizzle_w(
        np.ascontiguousarray(wout_pad.T).astype(np.float16))
    consts_cols.append(_vec_cols(bout_pad))
    shared["consts"] = np.ascontiguousarray(np.concatenate(consts_cols, axis=1))

    in_maps = []
    for c in range(NCORES):
        m = dict(shared)
        m["xt"] = np.ascontiguousarray(xt_all[:, c * BC:(c + 1) * BC])
        in_maps.append(m)
    return in_maps


def kernel(x, Wi, bi, Wq, bq, Wk, bk, Wv, bv, Wo, bo, conv_w, conv_b,
           W1, b1, ln1_g, ln1_b, W2, b2, ln2_g, ln2_b, Wout, bout):
    # Wq/bq/Wk/bk/conv_w/conv_b are mathematically dead: the model's internal
    # sequence length is 1, so softmax over one key is exactly 1.0 and the
    # attention scores never affect the output.
    global _cached, last_results
    fast = (not np.any(np.asarray(ln1_b)) and not np.any(np.asarray(b2)))
    if _cached is None:
        _cached = {}
    if fast not in _cached:
        _cached[fast] = _build(fast=fast)
    nc = _cached[fast]

    in_maps = _prep(x, Wi, bi, Wv, bv, Wo, bo, W1, b1, ln1_g, ln1_b,
                    W2, b2, ln2_g, ln2_b, Wout, bout, fast=fast)
    res = run_bass_kernel_spmd(nc, in_maps, core_ids=list(range(NCORES)))
    last_results = res
    out_t = np.concatenate([r["outT"] for r in res.results], axis=1)  # [CP, B]
    return np.ascontiguousarray(out_t[:C].T.astype(np.float32))  # [B, C] fp32


# revision 33
# speedup vs baseline: 1.0011x; 1.0011x over previous
"""Trainium2 Bass kernel for nn_EnhancedBioKANModel (dense_transformer).

Model (B=4096, IN=3072, D=2048, C=1000, 3 blocks), with the key
mathematical simplifications:

1. The internal sequence length is 1, so attention's softmax over a single
   key is identically 1.0 and the whole score/top-k/conv machinery cancels:
       attn(h) = (h @ Wv^T + bv) @ Wo^T + bo
2. That attention is then a purely linear map, so the residual attention
   block folds into ONE matrix host-side:
       h <- h + attn(h) = (I + Wo Wv) h + (Wo bv + bo) = Wff h + bf
3. The MLP branch is decoupled from the attention output by pre-multiplying
   host-side:  W1A = W1c @ Wff, so  s = W1c h_nxt == W1A h + const.
   Per layer the device then runs   W1A -> W2 -> Wff   and the whole
   LayerNorm scale chain + residual applies overlap with Wff's PE work
   (no PE idle at layer boundaries).

Strategy: pure data-parallel over batch (512 rows/core on 8 cores),
feature-major activation layout [D partitions, 512 batch free],
host-pre-transposed fp16 weights (halves HBM traffic vs fp32 — the fp32
version is DMA-bound at ~290 GB/s/core), fp16 activations (matmul is
1 col/cycle for fp16 and fp32r alike; PSUM accumulates fp32), LayerNorm
means folded into host-centered W1/W2, variance via ScalarE-square + PE
ones-reduction, per-batch 1/std broadcast via K=1 matmul, PE warm-up
burst at start so the HAM clock gate is at 8/8 when the real GEMM
stream begins.
"""
from contextlib import ExitStack

import numpy as np

import concourse.bacc as bacc
import concourse.mybir as mybir
import concourse.tile as tile
from concourse.bass_utils import run_bass_kernel_spmd

F32 = mybir.dt.float32
F16 = mybir.dt.float16
AF = mybir.ActivationFunctionType
ALU = mybir.AluOpType

NCORES = 8
B, IN, D, DD, C = 4096, 3072, 2048, 4096, 1000
CP = 1024          # padded num_classes
BC = B // NCORES   # 512 batch per core
NB = 3
EPS = 1e-5
MG = 2             # m-chunks per psum group (2 -> zero group-boundary stall)
KB = 8             # k-chunks per weight DMA (512 KiB transfers, 4 KiB lines)
NWARM = 40         # PE warm-up matmuls: cover the DMA-ring ramp (~8-28us)
SQR = 16           # sq chunks pre-reduced on DVE per variance matmul

# consts packing (columns of [128, NCOL] fp32), per layer:
#   bf(16) b1c(32) g1(32) beta1(32) b2c(16) g2(16) beta2(16)
_LAYER_COLS = 160
_NCOL = 16 + NB * _LAYER_COLS + CP // 128

_cached = None
last_results = None


def _build(fast=True):
    nc = bacc.Bacc(trn_type="TRN2")

    def wparam(name, K, M):
        # swizzled: [M/256 groups, K/(128*KB) blocks, 128 partitions, KB*256]
        return nc.declare_dram_parameter(
            name, [M // (MG * 128), K // (128 * KB), 128, KB * MG * 128],
            F16, isOutput=False)

    xt = nc.declare_dram_parameter("xt", [IN, BC], F16, isOutput=False)
    wit = wparam("wit", IN, D)
    w1t = [wparam(f"w1t{l}", D, DD) for l in range(NB)]
    w2t = [wparam(f"w2t{l}", DD, D) for l in range(NB)]
    wfft = [wparam(f"wfft{l}", D, D) for l in range(NB)]
    woutt = wparam("woutt", D, CP)
    consts = nc.declare_dram_parameter("consts", [128, _NCOL], F32, isOutput=False)
    out_t = nc.declare_dram_parameter("outT", [CP, BC], F16, isOutput=True)

    with tile.TileContext(nc) as tc, ExitStack() as ctx:
        wpool = ctx.enter_context(tc.tile_pool(name="w", bufs=4))
        xpool = ctx.enter_context(tc.tile_pool(name="x", bufs=1))
        spool = ctx.enter_context(tc.tile_pool(name="s", bufs=1))
        hapool = ctx.enter_context(tc.tile_pool(name="ha", bufs=1))
        hbpool = ctx.enter_context(tc.tile_pool(name="hb", bufs=1))
        hnpool = ctx.enter_context(tc.tile_pool(name="hn", bufs=1))
        upool = ctx.enter_context(tc.tile_pool(name="u", bufs=1))
        tpool = ctx.enter_context(tc.tile_pool(name="t", bufs=4))
        sqpool = ctx.enter_context(tc.tile_pool(name="sq", bufs=3))
        opool = ctx.enter_context(tc.tile_pool(name="o", bufs=2))
        statpool = ctx.enter_context(tc.tile_pool(name="stat", bufs=1))
        singles = ctx.enter_context(tc.tile_pool(name="singles", bufs=1))
        psum = ctx.enter_context(tc.tile_pool(name="psum", bufs=6, space="PSUM"))
        pstat = ctx.enter_context(tc.tile_pool(name="pstat", bufs=1, space="PSUM"))
        pbc = ctx.enter_context(tc.tile_pool(name="pbc", bufs=1, space="PSUM"))

        # ---- optional PE warm-up (NWARM matmuls on a memset tile; with the
        # x/weight DMAs on parallel HWDGE rings the real Wi stream starts
        # early enough to warm the HAM clock gate itself).
        if NWARM:
            wu_w = singles.tile([128, 128], F16)
            nc.vector.memset(wu_w, 0.5)
            wu_ps = pbc.tile([128, 128], F32, name="wu", tag="bc")
            for i in range(NWARM):
                nc.tensor.matmul(wu_ps[:, :], lhsT=wu_w[:, :], rhs=wu_w[:, :],
                                 start=True, stop=True)

        # ---- constants ----
        cst = singles.tile([128, _NCOL], F32)
        nc.gpsimd.dma_start(out=cst, in_=consts[:, :])

        ones_f = singles.tile([128, 1], F32)
        nc.vector.memset(ones_f, 1.0)
        ones_col = singles.tile([128, 1], F16)
        nc.vector.tensor_copy(ones_col[:, :], ones_f[:, :])
        ones_row_f = singles.tile([1, 128], F32)
        nc.vector.memset(ones_row_f, 1.0)
        ones_row = singles.tile([1, 128], F16)
        nc.vector.tensor_copy(ones_row[:, :], ones_row_f[:, :])
        eps_sb = singles.tile([1, 1], F32)
        nc.vector.memset(eps_sb, EPS)
        eps2_sb = singles.tile([1, 1], F32)
        nc.vector.memset(eps2_sb, EPS * EPS)

        col = [0]

        def take_cols(n):
            c0 = col[0]
            col[0] += n
            return cst[:, c0:c0 + n]

        bi_v = take_cols(D // 128)
        layer_cols = []
        for l in range(NB):
            layer_cols.append(dict(
                bf=take_cols(D // 128),
                b1c=take_cols(DD // 128), g1=take_cols(DD // 128),
                beta1=take_cols(DD // 128), b2c=take_cols(D // 128),
                g2=take_cols(D // 128), beta2=take_cols(D // 128)))
        bout_v = take_cols(CP // 128)

        # fp16 copies of the per-layer apply constants (keeps the DVE
        # applies in all-16-bit mode)
        csth = singles.tile([128, NB * 2 * (D // 128)], F16)
        for l in range(NB):
            o = l * 2 * (D // 128)
            nc.vector.tensor_copy(csth[:, o:o + D // 128], layer_cols[l]["g2"])
            nc.vector.tensor_copy(csth[:, o + D // 128:o + 2 * (D // 128)],
                                  layer_cols[l]["beta2"])

        def g2h(l):
            return csth[:, l * 2 * (D // 128):l * 2 * (D // 128) + D // 128]

        def beta2h(l):
            o = l * 2 * (D // 128) + D // 128
            return csth[:, o:o + D // 128]

        # Deferred variance-reduction matmuls: the PE executes its queue in
        # order, so a stat matmul emitted right at its group's end stalls the
        # PE ~0.5us on the ScalarE Square. Instead the Square is emitted at
        # evac time and the ones-matmul is queued here, flushed a couple of
        # k-chunks into the NEXT psum group (by which time sq is ready).
        pending_stats = []

        def flush_stats():
            while pending_stats:
                ps_var, sq, m, mc = pending_stats.pop(0)
                nc.tensor.matmul(ps_var[:, :], lhsT=ones_col[:, :],
                                 rhs=sq[:, :], start=(m == 0),
                                 stop=(m == mc - 1))

        # ---- generic GEMM driver (swizzled fp16 weights, 512 KiB DMAs) ----
        def gemm(wt_dram, k_chunks, m_chunks, rhs_fn, evac_fn, label,
                 first=False, mid=None):
            """psum[m] = sum_k WT[k,m].T @ rhs(k); evac_fn(m, psum).
            mid() is emitted into the PE stream after group 1's k-loop."""
            n_groups = m_chunks // MG
            nkb = k_chunks // KB
            for mg in range(n_groups):
                mlo = mg * MG
                pss = [psum.tile([128, BC], F32, name=f"ps_{label}_{mlo + i}",
                                 tag="ps") for i in range(MG)]
                for kbi in range(nkb):
                    w_sb = wpool.tile([128, KB, MG * 128], F16,
                                      name=f"w_{label}_{mg}_{kbi}", tag="w")
                    weng = nc.gpsimd if (first and mg % 2 == 1) else nc.sync
                    weng.dma_start(out=w_sb, in_=wt_dram[mg, kbi])
                    for kk in range(KB):
                        k = kbi * KB + kk
                        rhs = rhs_fn(k)
                        for i in range(MG):
                            nc.tensor.matmul(
                                pss[i][:, :],
                                lhsT=w_sb[:, kk, i * 128:(i + 1) * 128],
                                rhs=rhs, start=(k == 0),
                                stop=(k == k_chunks - 1))
                        if kbi == 1 and kk == 1:
                            flush_stats()
                        if mid is not None and mg == 1 and kbi == nkb - 1 \
                                and kk == KB - 1:
                            mid()
                for i in range(MG):
                    evac_fn(mlo + i, pss[i])

        # ---- LN helpers ----
        sq_quad = []

        def sq_reduce(ps, bias_col, m, m_chunks, ps_var, label):
            """ps_var[1,BC] += colsum of (psum + bias)^2. Square on ScalarE;
            SQR chunks are tree-summed on DVE so only m_chunks/SQR
            ones-matmuls hit the PE (each costs ~0.4us of PE time);
            those matmuls are deferred into a later psum group's k-loop."""
            sq = sqpool.tile([128, BC], F16, name=f"sq_{label}_{m}", tag="sq",
                             bufs=SQR + 4)
            nc.scalar.activation(out=sq[:, :], in_=ps[:, :], func=AF.Square,
                                 bias=bias_col, scale=1.0)
            sq_quad.append(sq)
            if len(sq_quad) == SQR:
                level = list(sq_quad)
                sq_quad.clear()
                gen = 0
                while len(level) > 1:
                    nxt = []
                    for j in range(0, len(level), 2):
                        t = sqpool.tile([128, BC], F16,
                                        name=f"sqr_{label}_{m}_{gen}_{j}",
                                        tag=f"sqr{gen}_{j}", bufs=2)
                        nc.vector.scalar_tensor_tensor(
                            out=t[:, :], in0=level[j][:, :], scalar=0.0,
                            in1=level[j + 1][:, :], op0=ALU.add, op1=ALU.add)
                        nxt.append(t)
                    level = nxt
                    gen += 1
                q = m // SQR
                pending_stats.append((ps_var, level[0], q, m_chunks // SQR))

        def rsqrt_bcast(v_ap, label):
            """[1,BC] f32 v -> broadcast [128,BC] fp16 of 1/sqrt(v) in SBUF."""
            inv = statpool.tile([1, BC], F32, name=f"inv_{label}", tag="inv")
            try:
                nc.vector.reciprocal_approx_fast(out=inv[:, :], in_=v_ap)
            except AttributeError:
                with nc.allow_low_precision(reason="LN scale"):
                    nc.vector.reciprocal(out=inv[:, :], in_=v_ap)
            invh = statpool.tile([1, BC], F16, name=f"invh_{label}", tag="invh")
            nc.vector.tensor_copy(invh[:, :], inv[:, :])
            ibc = pbc.tile([128, BC], F32, name=f"ibc_{label}", tag="bc")
            nc.tensor.matmul(ibc[:, :], lhsT=ones_row[:, :], rhs=invh[:, :],
                             start=True, stop=True)
            ibc_sb = statpool.tile([128, BC], F16, name=f"ibcsb_{label}",
                                   tag="ibcsb", bufs=2)
            nc.scalar.activation(out=ibc_sb[:, :], in_=ibc[:, :],
                                 func=AF.Identity, bias=0.0, scale=1.0)
            return ibc_sb

        # ---- phase 1: h = relu(Wi @ x + bi) ----
        x_sb = [xpool.tile([128, BC], F16, name=f"x{k}", tag="xs", bufs=IN // 128)
                for k in range(IN // 128)]
        x_loaded = [False] * (IN // 128)

        def x_chunk(k):
            if not x_loaded[k]:
                # ACT-ring HWDGE: keeps the x chunks off the SP ring that
                # streams the (much larger) weight blocks
                nc.scalar.dma_start(out=x_sb[k], in_=xt[k * 128:(k + 1) * 128, :])
                x_loaded[k] = True
            return x_sb[k][:, :]

        h_a = [hapool.tile([128, BC], F16, name=f"h_a{m}", tag="ha", bufs=16)
               for m in range(D // 128)]
        h_b = [hbpool.tile([128, BC], F16, name=f"h_b{m}", tag="hb", bufs=16)
               for m in range(D // 128)]
        hn = [hnpool.tile([128, BC], F16, name=f"hn{m}", tag="hn", bufs=16)
              for m in range(D // 128)]
        s_sb = [spool.tile([128, BC], F16, name=f"s{m}", tag="s", bufs=DD // 128)
                for m in range(DD // 128)]
        u_sb = [upool.tile([128, BC], F16, name=f"u{m}", tag="um", bufs=16)
                for m in range(D // 128)]

        def evac_h0(m, ps):
            nc.scalar.activation(out=h_a[m][:, :], in_=ps[:, :], func=AF.Relu,
                                 bias=bi_v[:, m:m + 1], scale=1.0)

        gemm(wit, IN // 128, D // 128, x_chunk, evac_h0, "wi", first=True)

        # ---- phase 2: layers ----
        h_cur = h_a
        h_new = h_b
        for l in range(NB):
            lc = layer_cols[l]

            # (a) s-branch first: shat = W1A h + b1eff  (W1A = W1c @ Wff);
            #     variance stats on the fly; fast path stores relu(shat)
            #     (LN1 1/std deferred through W2 into c12)
            ps_var1 = pstat.tile([1, BC], F32, name=f"pv1_{l}", tag="pv")

            def evac_s(m, ps, lc=lc, ps_var1=ps_var1, l=l):
                sq_reduce(ps, lc["b1c"][:, m:m + 1], m, DD // 128, ps_var1,
                          f"l1_{l}")
                nc.scalar.activation(out=s_sb[m][:, :], in_=ps[:, :],
                                     func=AF.Relu if fast else AF.Identity,
                                     bias=lc["b1c"][:, m:m + 1], scale=1.0)

            gemm(w1t[l], D // 128, DD // 128,
                 lambda k, h=h_cur: h[k][:, :], evac_s, f"w1{l}")

            if fast:
                # E = eps * std1^2 = eps*(pv1/DD + eps)
                e_t = statpool.tile([1, BC], F32, name=f"e_{l}", tag="e_t")
                nc.scalar.activation(out=e_t[:, :], in_=ps_var1[:, :],
                                     func=AF.Identity, bias=eps2_sb[:, :],
                                     scale=EPS / DD)
            else:
                # explicit LN1: s = relu(shat * g1 * inv1 + beta1).
                # pv1's final stat matmuls must precede the bcast matmul in
                # the in-order PE queue.
                flush_stats()
                v1 = statpool.tile([1, BC], F32, name=f"v1_{l}", tag="e_t")
                nc.scalar.activation(out=v1[:, :], in_=ps_var1[:, :],
                                     func=AF.Sqrt, bias=eps_sb[:, :],
                                     scale=1.0 / DD)
                ibc1_sb = rsqrt_bcast(v1[:, :], f"l1_{l}")
                for m in range(DD // 128):
                    t1 = tpool.tile([128, BC], F16, name=f"t1_{l}_{m}", tag="ta")
                    nc.vector.scalar_tensor_tensor(
                        out=t1[:, :], in0=s_sb[m][:, :],
                        scalar=lc["g1"][:, m:m + 1], in1=ibc1_sb[:, :],
                        op0=ALU.mult, op1=ALU.mult)
                    nc.scalar.activation(out=s_sb[m][:, :], in_=t1[:, :],
                                         func=AF.Relu,
                                         bias=lc["beta1"][:, m:m + 1],
                                         scale=1.0)

            # (b) u = W2c s + b2c (pre-centered, |g1| folded in); var stats
            ps_var2 = pstat.tile([1, BC], F32, name=f"pv2_{l}", tag="pv")

            def evac_u(m, ps, lc=lc, ps_var2=ps_var2, l=l):
                sq_reduce(ps, lc["b2c"][:, m:m + 1], m, D // 128, ps_var2,
                          f"l2_{l}")
                nc.scalar.activation(out=u_sb[m][:, :], in_=ps[:, :],
                                     func=AF.Identity,
                                     bias=lc["b2c"][:, m:m + 1], scale=1.0)

            gemm(w2t[l], DD // 128, D // 128,
                 lambda k, s=s_sb: s[k][:, :], evac_u, f"w2{l}")

            # (c) LN2 scale: c12 = 1/sqrt(pv2/D + E)  (fast: E = eps*std1^2,
            #     recovering the deferred LN1 scale; slow: E = eps).
            #     Emitted via the Wff gemm's `mid` hook so the bcast matmul
            #     enters the in-order PE queue after Wff group 1 — by then
            #     pv2 is complete and the DVE/ScalarE chain has run, so
            #     nothing stalls and the applies overlap the Wff GEMM.
            ibc2_holder = []

            def mid_c12(l=l, ps_var2=ps_var2):
                if fast:
                    v2 = statpool.tile([1, BC], F32, name=f"v2_{l}", tag="v2")
                    nc.vector.scalar_tensor_tensor(
                        out=v2[:, :], in0=ps_var2[:, :], scalar=1.0 / D,
                        in1=e_t[:, :], op0=ALU.mult, op1=ALU.add)
                    sr = statpool.tile([1, BC], F32, name=f"sr_{l}", tag="std")
                    nc.scalar.activation(out=sr[:, :], in_=v2[:, :],
                                         func=AF.Sqrt, bias=0.0, scale=1.0)
                else:
                    sr = statpool.tile([1, BC], F32, name=f"sr_{l}", tag="std")
                    nc.scalar.activation(out=sr[:, :], in_=ps_var2[:, :],
                                         func=AF.Sqrt, bias=eps_sb[:, :],
                                         scale=1.0 / D)
                ibc2_holder.append(rsqrt_bcast(sr[:, :], f"l2_{l}"))

            # (d) attention branch: hn = Wff h + bf  (identity folded in)
            def evac_att(m, ps, lc=lc):
                nc.scalar.activation(out=hn[m][:, :], in_=ps[:, :],
                                     func=AF.Identity,
                                     bias=lc["bf"][:, m:m + 1], scale=1.0)

            gemm(wfft[l], D // 128, D // 128,
                 lambda k, h=h_cur: h[k][:, :], evac_att, f"wff{l}",
                 mid=mid_c12)
            ibc2_sb = ibc2_holder[0]

            # (e) residual: h_new = hn + (u*g2)*c12 + beta2  (all-fp16 DVE,
            #     overlapped with the Wff GEMM / next-layer W1A)
            for m in range(D // 128):
                t = tpool.tile([128, BC], F16, name=f"t_{l}_{m}", tag="ta")
                nc.vector.scalar_tensor_tensor(
                    out=t[:, :], in0=u_sb[m][:, :],
                    scalar=g2h(l)[:, m:m + 1], in1=ibc2_sb[:, :],
                    op0=ALU.mult, op1=ALU.mult)
                nc.vector.scalar_tensor_tensor(
                    out=h_new[m][:, :], in0=t[:, :],
                    scalar=beta2h(l)[:, m:m + 1], in1=hn[m][:, :],
                    op0=ALU.add, op1=ALU.add)

            h_cur, h_new = h_new, h_cur

        # ---- phase 3: outT = Wout h + bout ----
        def evac_out(m, ps):
            o_sb = opool.tile([128, BC], F16, name=f"o{m}", tag="o")
            nc.scalar.activation(out=o_sb[:, :], in_=ps[:, :], func=AF.Identity,
                                 bias=bout_v[:, m:m + 1], scale=1.0)
            nc.sync.dma_start(out=out_t[m * 128:(m + 1) * 128, :], in_=o_sb[:, :])

        gemm(woutt, D // 128, CP // 128,
             lambda k, h=h_cur: h[k][:, :], evac_out, "wout")

    nc.compile()
    return nc


def _vec_cols(v):
    v = np.ascontiguousarray(v, dtype=np.float32)
    return v.reshape(-1, 128).T  # [128, L/128]


def _swizzle_w(wt):
    """[K, M] f16 -> [M/256, K/(128*KB), 128, KB*256] so one DMA moves a
    contiguous 512 KiB block with 4 KiB per-partition lines."""
    K, M = wt.shape
    MW = MG * 128
    a = wt.reshape(K // (128 * KB), KB, 128, M // MW, MW)
    return np.ascontiguousarray(a.transpose(3, 0, 2, 1, 4).reshape(
        M // MW, K // (128 * KB), 128, KB * MW))


def _prep(x, Wi, bi, Wv, bv, Wo, bo, W1, b1, ln1_g, ln1_b,
          W2, b2, ln2_g, ln2_b, Wout, bout, fast=True):
    f = np.float64
    xt_all = np.ascontiguousarray(np.asarray(x, np.float16).T)   # [IN, B]
    wit = _swizzle_w(np.asarray(Wi, np.float32).T.astype(np.float16))

    shared = {"wit": wit, "consts": None}
    consts_cols = [_vec_cols(np.asarray(bi, np.float32))]
    eye = np.eye(D, dtype=f)
    for l in range(NB):
        Wvl = np.asarray(Wv[l], f)
        Wol = np.asarray(Wo[l], f)
        bvl = np.asarray(bv[l], f)
        bol = np.asarray(bo[l], f)
        Wff = eye + Wol @ Wvl
        bf = Wol @ bvl + bol
        W1l = np.asarray(W1[l], f)
        W2l = np.asarray(W2[l], f)
        W1c = W1l - W1l.mean(axis=0, keepdims=True)
        W2c = W2l - W2l.mean(axis=0, keepdims=True)
        b1l = np.asarray(b1[l], f)
        b2l = np.asarray(b2[l], f)
        b1cl = b1l - b1l.mean()
        if fast:
            # fold sign(g1) into W1 rows (so relu-at-evac is valid) and
            # |g1| into W2 columns; requires ln1_b == 0.
            g1l = np.asarray(ln1_g[l], f)
            sgn = np.where(g1l < 0, -1.0, 1.0)
            W1c = W1c * sgn[:, None]
            b1cl = b1cl * sgn
            W2c = W2c * np.abs(g1l)[None, :]
        # decouple the s-branch from the attention output: s = W1c h_nxt
        # == (W1c Wff) h + (W1c bf + b1c)
        W1A = W1c @ Wff
        b1eff = W1c @ bf + b1cl
        shared[f"wfft{l}"] = _swizzle_w(
            np.ascontiguousarray(Wff.T).astype(np.float16))
        shared[f"w1t{l}"] = _swizzle_w(
            np.ascontiguousarray(W1A.T).astype(np.float16))
        shared[f"w2t{l}"] = _swizzle_w(
            np.ascontiguousarray(W2c.T).astype(np.float16))
        consts_cols += [
            _vec_cols(bf),
            _vec_cols(b1eff), _vec_cols(np.asarray(ln1_g[l], np.float32)),
            _vec_cols(np.asarray(ln1_b[l], np.float32)),
            _vec_cols(b2l - b2l.mean()),
            _vec_cols(np.asarray(ln2_g[l], np.float32)),
            _vec_cols(np.asarray(ln2_b[l], np.float32))]
    wout_pad = np.zeros((CP, D), np.float32)
    wout_pad[:C] = np.asarray(Wout, np.float32)
    bout_pad = np.zeros((CP,), np.float32)
    bout_pad[:C] = np.asarray(bout, np.float32)
    shared["woutt"] = _swizzle_w(
        np.ascontiguousarray(wout_pad.T).astype(np.float16))
    consts_cols.append(_vec_cols(bout_pad))
    shared["consts"] = np.ascontiguousarray(np.concatenate(consts_cols, axis=1))

    in_maps = []
    for c in range(NCORES):
        m = dict(shared)
        m["xt"] = np.ascontiguousarray(xt_all[:, c * BC:(c + 1) * BC])
        in_maps.append(m)
    return in_maps


def kernel(x, Wi, bi, Wq, bq, Wk, bk, Wv, bv, Wo, bo, conv_w, conv_b,
           W1, b1, ln1_g, ln1_b, W2, b2, ln2_g, ln2_b, Wout, bout):
    # Wq/bq/Wk/bk/conv_w/conv_b are mathematically dead: the model's internal
    # sequence length is 1, so softmax over one key is exactly 1.0 and the
    # attention scores never affect the output.
    global _cached, last_results
    fast = (not np.any(np.asarray(ln1_b)) and not np.any(np.asarray(b2)))
    if _cached is None:
        _cached = {}
    if fast not in _cached:
        _cached[fast] = _build(fast=fast)
    nc = _cached[fast]

    in_maps = _prep(x, Wi, bi, Wv, bv, Wo, bo, W1, b1, ln1_g, ln1_b,
                    W2, b2, ln2_g, ln2_b, Wout, bout, fast=fast)
    res = run_bass_kernel_spmd(nc, in_maps, core_ids=list(range(NCORES)))
    last_results = res
    out_t = np.concatenate([r["outT"] for r in res.results], axis=1)  # [CP, B]
    return np.ascontiguousarray(out_t[:C].T.astype(np.float32))  # [B, C] fp32
